# revision 1
# baseline (speedup 1.0000x reference)
"""Causal self-attention (B=4, T=4096, D=H=1024, fp32) on 8 Trainium2 cores.

Sharding: 2 cores per batch element. Within a batch, the 32 query tiles of
128 rows are interleaved between the 2 cores (core `pair` p takes global
q-tiles p, p+2, p+4, ...), which balances the causal-attention work exactly.
Each core computes the full K/V projection for its batch (replicated between
the 2 cores of a batch), then flash-style attention over its 16 q-tiles.

Numerics: x and the weights are cast to bf16 on the host; all matmuls run
bf16 with fp32 PSUM accumulation. Softmax skips max-subtraction (scores are
~N(0,1) after the 1/32 scale so exp stays in a safe fp32 range); exp runs on
ScalarE in fp32, probabilities are stored bf16, and the final normalization
is fp32. Measured error vs the fp32 reference: ~0.4% scale-relative absmax.
"""

import numpy as np

B, T, D, H = 4, 4096, 1024, 1024
P = 128
NCORES = 8


DEFAULT_CFG = dict(
    phases="AB",
    xpose="pe",           # "pe": PE transposes; "dma": xbar-transpose loads of x^T
    wq_top=False,         # preload Wq before phase A
    pa_xb_bufs=8, pa_xt_bufs=1,
    pa_pst_bufs=2, pa_psk_bufs=2, pa_psv_bufs=2,
    pb_xb_bufs=8, pb_xt_bufs=1, pb_qt_bufs=1,
    pb_p_bufs=3, pb_pt_bufs=2, pb_ob_bufs=2,
    pb_pp_bufs=3, pb_ps_bufs=3, pb_po_bufs=1,
    s_ahead=2,
)


def _emit(ctx, tc, xq, xkv, wq, wk, wv, maskt, ident, outp, T_kv, n_qt, cfg):
    import concourse.mybir as mybir

    nc = tc.nc
    f32 = mybir.dt.float32
    bf16 = mybir.dt.bfloat16
    Copy = mybir.ActivationFunctionType.Copy
    Exp = mybir.ActivationFunctionType.Exp
    AX = mybir.AxisListType.X
    SCALE = 1.0 / 32.0  # 1/sqrt(H)

    NKB = T_kv // 128     # kv 128-blocks
    NKC_A = T_kv // 512   # phase-A 512-row projection chunks
    NSC = n_qt // 4       # 512-row query superchunks
    if "A" not in cfg["phases"]:
        NKC_A = 0
    if "B" not in cfg["phases"]:
        NSC = 0

    const = ctx.enter_context(tc.tile_pool(name="const", bufs=1))
    persist = ctx.enter_context(tc.tile_pool(name="persist", bufs=1))

    id_sb = const.tile([P, P], bf16, tag="ident")
    nc.sync.dma_start(out=id_sb, in_=ident)
    mask_sb = const.tile([P, 256], bf16, tag="mask")
    nc.sync.dma_start(out=mask_sb, in_=maskt)

    # K^T laid out [h%128, h//128, t]; V laid out [t%128, t//128, h]
    KT = persist.tile([P, 8, T_kv], bf16, tag="KT")
    V = persist.tile([P, NKB, 1024], bf16, tag="V")

    def load_weight(wdram, wsb):
        # DRAM [1024,1024] bf16 -> SBUF [128, 8, 1024] (d = dc*128 + p)
        for dc in range(8):
            nc.sync.dma_start(out=wsb[:, dc, :], in_=wdram[dc * P:(dc + 1) * P, :])

    dma_xpose = cfg["xpose"] == "dma"
    wq_top = cfg["wq_top"]
    if wq_top:
        wq_sb_top = persist.tile([P, 8, 1024], bf16, tag="wq")
        load_weight(wq, wq_sb_top)

    def load_xt(xt, xsrc, r0, ps_pool, xb_pool, xb_tag):
        """Fill xt[:, dc, :] = x[r0:r0+512, dc*128:(dc+1)*128]^T for all dc."""
        if dma_xpose:
            for dc in range(8):
                nc.sync.dma_start_transpose(
                    out=xt[:, dc, :],
                    in_=xsrc[r0:r0 + 512, dc * P:(dc + 1) * P])
            return
        xbs = []
        for i in range(4):
            xb = xb_pool.tile([P, 1024], bf16, tag=xb_tag)
            nc.sync.dma_start(out=xb, in_=xsrc[r0 + i * P: r0 + (i + 1) * P, :])
            xbs.append(xb)
        for hf in range(2):
            for dc in range(8):
                tp = ps_pool.tile([P, 256], bf16, tag="pp")
                for i in range(2):
                    nc.tensor.transpose(
                        tp[:, i * P:(i + 1) * P],
                        xbs[hf * 2 + i][:, dc * P:(dc + 1) * P], id_sb)
                nc.scalar.activation(
                    out=xt[:, dc, hf * 256:(hf + 1) * 256], in_=tp, func=Copy)

    from contextlib import ExitStack as _ES

    # ---------------- Phase A: K/V projection over all kv rows ----------------
    with _ES() as pa:
        wpool = pa.enter_context(tc.tile_pool(name="pa_w", bufs=1))
        xtpool = pa.enter_context(tc.tile_pool(name="pa_xt", bufs=cfg["pa_xt_bufs"]))
        psA_k = pa.enter_context(
            tc.tile_pool(name="pa_psk", bufs=cfg["pa_psk_bufs"], space="PSUM"))
        psA_v = pa.enter_context(
            tc.tile_pool(name="pa_psv", bufs=cfg["pa_psv_bufs"], space="PSUM"))
        xbp = psA_t = None
        if not dma_xpose:
            xbp = pa.enter_context(
                tc.tile_pool(name="pa_xb", bufs=cfg["pa_xb_bufs"]))
            psA_t = pa.enter_context(
                tc.tile_pool(name="pa_pst", bufs=cfg["pa_pst_bufs"], space="PSUM"))
        wk_sb = wpool.tile([P, 8, 1024], bf16, tag="wk")
        wv_sb = wpool.tile([P, 8, 1024], bf16, tag="wv")
        load_weight(wk, wk_sb)
        load_weight(wv, wv_sb)

        for c in range(NKC_A):
            t0 = c * 512
            xt = xtpool.tile([P, 8, 512], bf16, tag="xt")
            load_xt(xt, xkv, t0, psA_t, xbp, "xb")
            # K^T_[h, t0:t0+512] = Wk^T @ x^T
            for hc in range(8):
                kp = psA_k.tile([P, 512], f32, tag="kp")
                for dc in range(8):
                    nc.tensor.matmul(
                        kp, lhsT=wk_sb[:, dc, hc * P:(hc + 1) * P],
                        rhs=xt[:, dc, :], start=(dc == 0), stop=(dc == 7))
                nc.vector.tensor_copy(out=KT[:, hc, t0:t0 + 512], in_=kp)
            # V_[t0+i*128, :] = x @ Wv
            for i in range(4):
                vp = psA_v.tile([P, 1024], f32, tag="vp")
                for dc in range(8):
                    for nb in range(2):
                        nc.tensor.matmul(
                            vp[:, nb * 512:(nb + 1) * 512],
                            lhsT=xt[:, dc, i * P:(i + 1) * P],
                            rhs=wv_sb[:, dc, nb * 512:(nb + 1) * 512],
                            start=(dc == 0), stop=(dc == 7))
                nc.vector.tensor_copy(out=V[:, t0 // P + i, :], in_=vp)

    # ---------------- Phase B: Q projection + attention ----------------
    with _ES() as pb_es:
        ec = pb_es.enter_context
        xtq_p = ec(tc.tile_pool(name="pb_xt", bufs=cfg["pb_xt_bufs"]))
        qt_p = ec(tc.tile_pool(name="pb_qt", bufs=cfg["pb_qt_bufs"]))
        pb_p = ec(tc.tile_pool(name="pb_p", bufs=cfg["pb_p_bufs"]))
        pt_p = ec(tc.tile_pool(name="pb_pt", bufs=cfg["pb_pt_bufs"]))
        sums_p = ec(tc.tile_pool(name="pb_sums", bufs=2))
        ob_p = ec(tc.tile_pool(name="pb_ob", bufs=cfg["pb_ob_bufs"]))
        ps_pp = ec(tc.tile_pool(name="pb_pp", bufs=cfg["pb_pp_bufs"], space="PSUM"))
        ps_s = ec(tc.tile_pool(name="pb_ps", bufs=cfg["pb_ps_bufs"], space="PSUM"))
        ps_o = ec(tc.tile_pool(name="pb_po", bufs=cfg["pb_po_bufs"], space="PSUM"))
        xbq_p = None
        if not dma_xpose:
            xbq_p = ec(tc.tile_pool(name="pb_xb", bufs=cfg["pb_xb_bufs"]))
        if wq_top:
            wq_sb = wq_sb_top
        else:
            wqp = ec(tc.tile_pool(name="pb_w", bufs=1))
            wq_sb = wqp.tile([P, 8, 1024], bf16, tag="wq")
            load_weight(wq, wq_sb)

        for sc in range(NSC):
            # Q^T for this superchunk: [h%128, h//128, 512 local q]
            xtq = xtq_p.tile([P, 8, 512], bf16, tag="xtq")
            load_xt(xtq, xq, sc * 512, ps_pp, xbq_p, "xbq")
            qt = qt_p.tile([P, 8, 512], bf16, tag="qt")
            for hc in range(8):
                qp = ps_pp.tile([P, 512], f32, tag="pp")
                for dc in range(8):
                    nc.tensor.matmul(
                        qp, lhsT=wq_sb[:, dc, hc * P:(hc + 1) * P],
                        rhs=xtq[:, dc, :], start=(dc == 0), stop=(dc == 7))
                nc.vector.tensor_copy(out=qt[:, hc, :], in_=qp)

            for o in range(4):
                j = sc * 4 + o
                nch = j + 1
                sums = sums_p.tile([P, 16], f32, tag="sums")
                op = ps_o.tile([P, 1024], f32, tag="op")

                def s_mm(c):
                    sp = ps_s.tile([P, 256], f32, tag="sp")
                    for hc in range(8):
                        nc.tensor.matmul(
                            sp, lhsT=qt[:, hc, o * P:(o + 1) * P],
                            rhs=KT[:, hc, c * 256:(c + 1) * 256],
                            start=(hc == 0), stop=(hc == 7))
                    return sp

                def softmax(c, sp):
                    pb = pb_p.tile([P, 256], bf16, tag="pb")
                    if c < nch - 1:
                        nc.scalar.activation(out=pb, in_=sp, func=Exp,
                                             scale=SCALE, accum_out=sums[:, c:c + 1])
                    else:
                        nc.scalar.activation(out=pb, in_=sp, func=Exp, scale=SCALE)
                        nc.vector.tensor_mul(pb, pb, mask_sb)
                        nc.vector.reduce_sum(out=sums[:, c:c + 1], in_=pb, axis=AX)
                    return pb

                def pv(c, pb):
                    ptp = ps_pp.tile([P, 256], bf16, tag="pp")
                    nc.tensor.transpose(ptp[:, 0:P], pb[:, 0:P], id_sb)
                    nc.tensor.transpose(ptp[:, P:256], pb[:, P:256], id_sb)
                    pt = pt_p.tile([P, 256], bf16, tag="pt")
                    nc.vector.tensor_copy(out=pt, in_=ptp)
                    for kl in range(2):
                        kb = c * 2 + kl
                        for nb in range(2):
                            nc.tensor.matmul(
                                op[:, nb * 512:(nb + 1) * 512],
                                lhsT=pt[:, kl * P:(kl + 1) * P],
                                rhs=V[:, kb, nb * 512:(nb + 1) * 512],
                                start=(c == 0 and kl == 0),
                                stop=(c == nch - 1 and kl == 1))

                ahead = cfg["s_ahead"]
                sps, pbs = {}, {}
                for c in range(min(ahead, nch)):
                    sps[c] = s_mm(c)
                    pbs[c] = softmax(c, sps[c])
                for c in range(nch):
                    pv(c, pbs[c])
                    if c + ahead < nch:
                        sps[c + ahead] = s_mm(c + ahead)
                        pbs[c + ahead] = softmax(c + ahead, sps[c + ahead])

                tot = sums_p.tile([P, 1], f32, tag="tot")
                nc.vector.reduce_sum(out=tot, in_=sums[:, 0:nch], axis=AX)
                rec = sums_p.tile([P, 1], f32, tag="rec")
                nc.vector.reciprocal(out=rec, in_=tot)
                ob = ob_p.tile([P, 1024], f32, tag="ob")
                nc.scalar.activation(out=ob, in_=op, func=Copy, scale=rec)
                nc.sync.dma_start(out=outp[j * P:(j + 1) * P, :], in_=ob)


def build_module(T_kv=T, n_qt=None, cfg=None):
    from contextlib import ExitStack
    import concourse.tile as tile
    import concourse.mybir as mybir
    from concourse import bacc

    if n_qt is None:
        n_qt = T_kv // 256
    full_cfg = dict(DEFAULT_CFG)
    if cfg:
        full_cfg.update(cfg)
    cfg = full_cfg
    dt = mybir.dt
    nc = bacc.Bacc("TRN2", target_bir_lowering=False, debug=False,
                   num_devices=NCORES)
    xq = nc.dram_tensor("xq", [n_qt * P, D], dt.bfloat16, kind="ExternalInput").ap()
    xkv = nc.dram_tensor("xkv", [T_kv, D], dt.bfloat16, kind="ExternalInput").ap()
    wq = nc.dram_tensor("wq", [D, H], dt.bfloat16, kind="ExternalInput").ap()
    wk = nc.dram_tensor("wk", [D, H], dt.bfloat16, kind="ExternalInput").ap()
    wv = nc.dram_tensor("wv", [D, H], dt.bfloat16, kind="ExternalInput").ap()
    maskt = nc.dram_tensor("maskt", [P, 256], dt.bfloat16, kind="ExternalInput").ap()
    ident = nc.dram_tensor("ident", [P, P], dt.bfloat16, kind="ExternalInput").ap()
    outp = nc.dram_tensor("outp", [n_qt * P, H], dt.float32, kind="ExternalOutput").ap()

    with tile.TileContext(nc) as tc:
        with ExitStack() as ctx:
            _emit(ctx, tc, xq, xkv, wq, wk, wv, maskt, ident, outp, T_kv, n_qt,
                  cfg)
    nc.compile()
    return nc


def host_inputs(x, Wq, Wk, Wv, T_kv=T, n_qt=None, n_batch=None):
    """Build the per-core input maps for run_bass_kernel_spmd."""
    import ml_dtypes
    bf = ml_dtypes.bfloat16
    if n_qt is None:
        n_qt = T_kv // 256
    if n_batch is None:
        n_batch = x.shape[0]
    eye = np.eye(P, dtype=np.float32).astype(bf)
    tril = np.tril(np.ones((P, P), np.float32))
    m = [np.concatenate([tril, np.zeros((P, P), np.float32)], 1).astype(bf),
         np.concatenate([np.ones((P, P), np.float32), tril], 1).astype(bf)]

    def make_masks(pair):
        return m[pair]
    xb = np.asarray(x, np.float32).astype(bf)
    wqb = np.asarray(Wq, np.float32).astype(bf)
    wkb = np.asarray(Wk, np.float32).astype(bf)
    wvb = np.asarray(Wv, np.float32).astype(bf)
    in_maps = []
    for c in range(NCORES):
        b, pair = (c // 2) % n_batch, c % 2
        qrows = np.concatenate(
            [xb[b, (2 * j + pair) * P:(2 * j + pair + 1) * P, :]
             for j in range(n_qt)], 0)
        in_maps.append({
            "xq": np.ascontiguousarray(qrows),
            "xkv": np.ascontiguousarray(xb[b]),
            "wq": wqb, "wk": wkb, "wv": wvb,
            "maskt": make_masks(pair), "ident": eye,
        })
    return in_maps


def gather_output(results, T_kv=T, n_qt=None, n_batch=B):
    if n_qt is None:
        n_qt = T_kv // 256
    out = np.empty((n_batch, T_kv, H), np.float32)
    for c in range(2 * n_batch):
        b, pair = c // 2, c % 2
        r = results[c]["outp"]
        for j in range(n_qt):
            out[b, (2 * j + pair) * P:(2 * j + pair + 1) * P, :] = \
                r[j * P:(j + 1) * P, :]
    return out


_NC_CACHE = {}


def kernel(x, Wq, Wk, Wv):
    from concourse.bass_utils import run_bass_kernel_spmd

    x = np.asarray(x, dtype=np.float32)
    Wq = np.asarray(Wq, dtype=np.float32)
    Wk = np.asarray(Wk, dtype=np.float32)
    Wv = np.asarray(Wv, dtype=np.float32)

    if "nc" not in _NC_CACHE:
        _NC_CACHE["nc"] = build_module()
    nc = _NC_CACHE["nc"]

    in_maps = host_inputs(x, Wq, Wk, Wv)
    res = run_bass_kernel_spmd(nc, in_maps, core_ids=list(range(NCORES)))
    return gather_output(res.results)



# revision 2
# speedup vs baseline: 1.1030x; 1.1030x over previous
"""Causal self-attention (B=4, T=4096, D=H=1024, fp32) on 8 Trainium2 cores.

Sharding: 2 cores per batch element (core pair). The 32 row-tiles of 128 are
interleaved between the 2 cores (core `pair` p owns global tiles p, p+2, ...),
balancing causal attention work.

Projections: each core projects Q and K for its OWN 2048 rows; the K^T halves
are exchanged between the pair with 2-rank AllGather collectives (2 chunks of
2MB, issued as early as possible) — K projection runs first so the collectives
overlap the V projection + early attention. V is projected redundantly by both
cores (local rows from xlt, peer rows from xrt) — duplicating V costs ~55us of
PE but removes 4MB from the slow (~40GB/s, serialized) collective path.

K^T / V live in SBUF in LOCAL|REMOTE halves: local block i (global 2i+pair) at
position i, peer block i at position 16+i. With tile-interleaved ownership the
attention loop is pair-independent in ADDRESSES: for q-tile j it uses local
blocks 0..j (diagonal mask triu on block j — same for both pairs) and remote
blocks 0..j, with the pair asymmetry absorbed by a host-supplied mask (block
j remote is all-zeros for pair 0, all-ones for pair 1). Only the collective
readback needs the runtime pair: two cond-predicated DMAs per transfer.

Attention is computed in S^T layout (scores[k, q]) so the probabilities come
out of the exp already transposed for the PV matmul — zero PE transposes (all
x transposition is done on the host). Softmax row sums come from a ones-column
matmul accumulated alongside PV.

Numerics: bf16 matmuls with fp32 PSUM accumulation; softmax skips
max-subtraction (scores ~N(0,1) after the 1/32 scale); exp on ScalarE,
probabilities stored bf16, final normalization fp32.
"""

import numpy as np

B, T, D, H = 4, 4096, 1024, 1024
P = 128
NCORES = 8
GROUPS = [[0, 1], [2, 3], [4, 5], [6, 7]]
TH = T // 2       # rows owned per core (2048)
NLB = TH // P     # local blocks per core (16)

DEFAULT_CFG = dict(
    n_cc=2,            # number of AllGather chunks for the K exchange
    xt_bufs=5,
    xtq_bufs=2, qt_bufs=1, wt_bufs=34, ob_bufs=2,
    ps_a_bufs=3, ps_o_bufs=2,
    pa_psk_bufs=2, pa_psv_bufs=2,
)


def _emit(ctx, tc, xlt, xrt, wq, wk, wv, masks, ones2, outp, cfg):
    import concourse.mybir as mybir
    from contextlib import ExitStack as _ES

    nc = tc.nc
    f32 = mybir.dt.float32
    bf16 = mybir.dt.bfloat16
    Copy = mybir.ActivationFunctionType.Copy
    Exp = mybir.ActivationFunctionType.Exp
    SCALE = 1.0 / 32.0  # 1/sqrt(H)

    n_cc = cfg["n_cc"]
    sub_per_cc = 4 // n_cc      # 512-row K pieces per collective chunk

    const = ctx.enter_context(tc.tile_pool(name="const", bufs=1))
    persist = ctx.enter_context(tc.tile_pool(name="persist", bufs=1))
    dram = ctx.enter_context(tc.tile_pool(name="dram", bufs=1, space="DRAM"))

    mask_sb = const.tile([P, 256], bf16, tag="mask")
    nc.sync.dma_start(out=mask_sb, in_=masks)
    ones_sb = const.tile([P, 2], bf16, tag="ones")
    nc.sync.dma_start(out=ones_sb, in_=ones2)

    # K^T [h%128, h//128, col]: cols [0:TH] local blocks, [TH:2TH] remote.
    # V [t%128, blk, h]: blks [0:16] local, [16:32] remote.
    KT = persist.tile([P, 8, T], bf16, tag="KT")
    V = persist.tile([P, T // P, H], bf16, tag="V")

    cc_ins, cc_outs = [], []
    for c in range(n_cc):
        cc_ins.append(dram.tile([P, sub_per_cc * 4096], bf16,
                                tag=f"cc_in{c}", name=f"cc_in{c}"))
        cc_outs.append(dram.tile([2 * P, sub_per_cc * 4096], bf16,
                                 tag=f"cc_out{c}", name=f"cc_out{c}"))

    is_even = (nc.sync.partition_id() % 2) == 0

    def load_weight(wdram, wsb, eng=None):
        # DRAM [1024,1024] bf16 -> SBUF [128, 8, 1024] (d = dc*128 + p)
        eng = eng or nc.sync
        for dc in range(8):
            eng.dma_start(out=wsb[:, dc, :], in_=wdram[dc * P:(dc + 1) * P, :])

    # ---------------- Phase A: projections + pair AllGather of K -------------
    with _ES() as pa:
        wpool = pa.enter_context(tc.tile_pool(name="pa_w", bufs=1))
        xtp = pa.enter_context(tc.tile_pool(name="pa_xt", bufs=cfg["xt_bufs"]))
        psk = pa.enter_context(
            tc.tile_pool(name="pa_psk", bufs=cfg["pa_psk_bufs"], space="PSUM"))
        psv = pa.enter_context(
            tc.tile_pool(name="pa_psv", bufs=cfg["pa_psv_bufs"], space="PSUM"))

        def load_xt(src, t0):
            xt = xtp.tile([P, 8, 512], bf16, tag="xt")
            for dc in range(8):
                nc.sync.dma_start(
                    out=xt[:, dc, :],
                    in_=src[dc * P:(dc + 1) * P, t0:t0 + 512])
            return xt

        xt0 = load_xt(xlt, 0)  # first x tile before the weights
        wk_sb = wpool.tile([P, 8, 1024], bf16, tag="wk")
        wv_sb = wpool.tile([P, 8, 1024], bf16, tag="wv")
        # split the weight loads across both HWDGE queues
        for dc in range(8):
            eng = nc.sync if dc < 4 else nc.scalar
            eng.dma_start(out=wk_sb[:, dc, :], in_=wk[dc * P:(dc + 1) * P, :])
        load_weight(wv, wv_sb, eng=nc.scalar)

        # K projection for my 2048 rows, written straight into KT local half,
        # staged out to the collective as each chunk completes. The xlt
        # tiles are kept (xt_bufs >= 5) and reused by the V projection.
        xts = [xt0]
        for s in range(4):
            t0 = s * 512
            xt = xts[s]
            if s < 3:
                xts.append(load_xt(xlt, t0 + 512))
            for hc in range(8):
                kp = psk.tile([P, 512], f32, tag="kp")
                for dc in range(8):
                    nc.tensor.matmul(
                        kp, lhsT=wk_sb[:, dc, hc * P:(hc + 1) * P],
                        rhs=xt[:, dc, :], start=(dc == 0), stop=(dc == 7))
                nc.vector.tensor_copy(out=KT[:, hc, t0:t0 + 512], in_=kp)
            c, sp_ = divmod(s, sub_per_cc)
            for hc in range(8):
                nc.sync.dma_start(
                    out=cc_ins[c][:, sp_ * 4096 + hc * 512:
                                  sp_ * 4096 + (hc + 1) * 512],
                    in_=KT[:, hc, t0:t0 + 512])
            if sp_ == sub_per_cc - 1:
                nc.gpsimd.collective_compute(
                    "AllGather",
                    mybir.AluOpType.bypass,
                    replica_groups=GROUPS,
                    ins=[cc_ins[c].opt()],
                    outs=[cc_outs[c].opt()],
                )

        # V projection: local rows (reusing the K loop's x tiles) then peer
        # rows, straight into V.
        def v_piece(xt, blk0):
            for i in range(4):
                vp = psv.tile([P, 1024], f32, tag="vp")
                for dc in range(8):
                    for nb in range(2):
                        nc.tensor.matmul(
                            vp[:, nb * 512:(nb + 1) * 512],
                            lhsT=xt[:, dc, i * P:(i + 1) * P],
                            rhs=wv_sb[:, dc, nb * 512:(nb + 1) * 512],
                            start=(dc == 0), stop=(dc == 7))
                nc.vector.tensor_copy(out=V[:, blk0 + i, :], in_=vp)

        xr0 = load_xt(xrt, 0)  # prefetch first peer piece during V-local
        for s in range(4):
            v_piece(xts[s], s * 4)
        for s in range(4):
            xr = xr0 if s == 0 else load_xt(xrt, s * 512)
            v_piece(xr, NLB + s * 4)

    def load_back(c):
        """Readback of the PEER's chunk-c K pieces into KT's remote half.

        cc_out rows [0:128] hold rank 0 (even core), [128:256] rank 1. The
        peer's rows depend on this core's parity, so each transfer is emitted
        twice with complementary cond predicates (skipped DMAs still bump
        their semaphores, so dependency tracking stays sound).
        """
        for s in range(sub_per_cc):
            col = TH + (c * sub_per_cc + s) * 512
            for hc in range(8):
                src_lo = cc_outs[c][P:2 * P, s * 4096 + hc * 512:
                                    s * 4096 + (hc + 1) * 512]
                src_hi = cc_outs[c][0:P, s * 4096 + hc * 512:
                                    s * 4096 + (hc + 1) * 512]
                nc.sync.dma_start(out=KT[:, hc, col:col + 512], in_=src_lo,
                                  cond=is_even)
                nc.sync.dma_start(out=KT[:, hc, col:col + 512], in_=src_hi,
                                  cond=(is_even == 0))

    # ---------------- Phase B: Q projection + S^T attention ------------------
    with _ES() as pb:
        ec = pb.enter_context
        wqp = ec(tc.tile_pool(name="pb_w", bufs=1))
        xtqp = ec(tc.tile_pool(name="pb_xtq", bufs=cfg["xtq_bufs"]))
        qtp = ec(tc.tile_pool(name="pb_qt", bufs=cfg["qt_bufs"]))
        wtp = ec(tc.tile_pool(name="pb_wt", bufs=cfg["wt_bufs"]))
        obp = ec(tc.tile_pool(name="pb_ob", bufs=cfg["ob_bufs"]))
        smp = ec(tc.tile_pool(name="pb_sm", bufs=2))
        ps_a = ec(tc.tile_pool(name="pb_psa", bufs=cfg["ps_a_bufs"], space="PSUM"))
        ps_o = ec(tc.tile_pool(name="pb_pso", bufs=cfg["ps_o_bufs"], space="PSUM"))
        ps_s = ec(tc.tile_pool(name="pb_pss", bufs=1, space="PSUM"))

        wq_sb = wqp.tile([P, 8, 1024], bf16, tag="wq")
        load_weight(wq, wq_sb, eng=nc.scalar)

        # first superchunk that needs collective chunk c (remote blocks
        # c*bpc.. ; superchunk sc uses remote blocks <= 4*sc+3)
        bpc = NLB // n_cc
        sc_of_chunk = {c: max(0, (c * bpc - 3 + 3) // 4) for c in range(n_cc)}

        for sc in range(4):
            # Q^T for this superchunk: [h%128, hc, 512]
            # scalar HWDGE queue: the sync queue may be backed up behind
            # collective readbacks, which must not starve the Q projection
            xtq = xtqp.tile([P, 8, 512], bf16, tag="xtq")
            for dc in range(8):
                nc.scalar.dma_start(
                    out=xtq[:, dc, :],
                    in_=xlt[dc * P:(dc + 1) * P, sc * 512:(sc + 1) * 512])
            for c in range(n_cc):
                if sc_of_chunk[c] == sc:
                    load_back(c)
            qt = qtp.tile([P, 8, 512], bf16, tag="qt")
            for hc in range(8):
                qp = ps_a.tile([P, 512], f32, tag="psa")
                for dc in range(8):
                    nc.tensor.matmul(
                        qp, lhsT=wq_sb[:, dc, hc * P:(hc + 1) * P],
                        rhs=xtq[:, dc, :], start=(dc == 0), stop=(dc == 7))
                nc.vector.tensor_copy(out=qt[:, hc, :], in_=qp)

            NL = 4 * sc + 4           # local/remote block count this superchunk
            sums = ps_s.tile([P, 8], f32, tag="sums")
            wtL, wtR = [], []

            def do_tile(o, sc=sc, sums=sums, wtL=wtL, wtR=wtR):
                """Mask diag blocks, then sums+PV accumulation for q-tile o."""
                j = 4 * sc + o
                qs = slice(o * P, (o + 1) * P)
                nc.vector.tensor_mul(
                    wtL[j][:, qs], wtL[j][:, qs], mask_sb[:, 0:P])
                nc.vector.tensor_mul(
                    wtR[j][:, qs], wtR[j][:, qs], mask_sb[:, P:2 * P])
                op = ps_o.tile([P, 1024], f32, tag="op")
                nslice = 2 * (j + 1)
                ns = 0
                for i in range(j + 1):
                    for wt_, vb in ((wtL, i), (wtR, NLB + i)):
                        sl = wt_[i][:, qs]
                        nc.tensor.matmul(
                            sums[:, 2 * o:2 * o + 2], lhsT=sl, rhs=ones_sb,
                            start=(ns == 0), stop=(ns == nslice - 1))
                        for nb in range(2):
                            nc.tensor.matmul(
                                op[:, nb * 512:(nb + 1) * 512], lhsT=sl,
                                rhs=V[:, vb, nb * 512:(nb + 1) * 512],
                                start=(ns == 0), stop=(ns == nslice - 1))
                        ns += 1
                rec = smp.tile([P, 1], f32, tag="rec")
                nc.vector.reciprocal(out=rec, in_=sums[:, 2 * o:2 * o + 1])
                ob = obp.tile([P, 1024], bf16, tag="ob")
                nc.scalar.activation(out=ob, in_=op, func=Copy, scale=rec)
                jj = 4 * sc + o
                # scalar HWDGE queue keeps outputs off the (waiting) sync queue
                nc.scalar.dma_start(out=outp[jj * P:(jj + 1) * P, :], in_=ob)

            for i in range(NL):
                rel = i - 4 * sc
                off = 0 if rel < 1 else P * rel
                for wt_, kcol in ((wtL, i * P), (wtR, TH + i * P)):
                    sp = ps_a.tile([P, 512], f32, tag="psa")
                    for hc in range(8):
                        nc.tensor.matmul(
                            sp[:, off:], lhsT=KT[:, hc, kcol:kcol + P],
                            rhs=qt[:, hc, off:], start=(hc == 0), stop=(hc == 7))
                    wt_kb = wtp.tile([P, 512], bf16, tag="wt")
                    nc.scalar.activation(out=wt_kb[:, off:], in_=sp[:, off:],
                                         func=Exp, scale=SCALE)
                    wt_.append(wt_kb)
                # tile o's PV is emitted one i-step late so its last exp
                # hides under the next step's score matmuls
                if rel >= 1:
                    do_tile(rel - 1)
            do_tile(3)


def build_module(cfg=None):
    from contextlib import ExitStack
    import concourse.tile as tile
    import concourse.mybir as mybir
    from concourse import bacc

    full_cfg = dict(DEFAULT_CFG)
    if cfg:
        full_cfg.update(cfg)
    cfg = full_cfg
    dt = mybir.dt
    nc = bacc.Bacc("TRN2", target_bir_lowering=False, debug=False,
                   num_devices=NCORES)
    xlt = nc.dram_tensor("xlt", [D, TH], dt.bfloat16, kind="ExternalInput").ap()
    xrt = nc.dram_tensor("xrt", [D, TH], dt.bfloat16, kind="ExternalInput").ap()
    wq = nc.dram_tensor("wq", [D, H], dt.bfloat16, kind="ExternalInput").ap()
    wk = nc.dram_tensor("wk", [D, H], dt.bfloat16, kind="ExternalInput").ap()
    wv = nc.dram_tensor("wv", [D, H], dt.bfloat16, kind="ExternalInput").ap()
    masks = nc.dram_tensor("masks", [P, 256], dt.bfloat16, kind="ExternalInput").ap()
    ones2 = nc.dram_tensor("ones2", [P, 2], dt.bfloat16, kind="ExternalInput").ap()
    outp = nc.dram_tensor("outp", [TH, H], dt.bfloat16, kind="ExternalOutput").ap()

    with tile.TileContext(nc) as tc:
        with ExitStack() as ctx:
            _emit(ctx, tc, xlt, xrt, wq, wk, wv, masks, ones2, outp, cfg)
    nc.compile()
    return nc


def host_inputs(x, Wq, Wk, Wv):
    """Build the per-core input maps for run_bass_kernel_spmd."""
    import ml_dtypes
    bf = ml_dtypes.bfloat16

    xb = np.asarray(x, np.float32).astype(bf)
    wqb = np.asarray(Wq, np.float32).astype(bf)
    wkb = np.asarray(Wk, np.float32).astype(bf)
    wvb = np.asarray(Wv, np.float32).astype(bf)

    tri = np.triu(np.ones((P, P), np.float32))  # keep k <= q
    m = [np.concatenate([tri, np.zeros((P, P), np.float32)], 1).astype(bf),
         np.concatenate([tri, np.ones((P, P), np.float32)], 1).astype(bf)]
    ones2 = np.ones((P, 2), np.float32).astype(bf)

    in_maps = []
    xTs = [np.ascontiguousarray(xb[b].T) for b in range(B)]  # [1024, 4096]
    gathers = []
    for pair in range(2):
        idx = np.concatenate(
            [np.arange((2 * j + pair) * P, (2 * j + pair + 1) * P)
             for j in range(NLB)])
        gathers.append(idx)
    for c in range(NCORES):
        b, pair = c // 2, c % 2
        xT = xTs[b]
        in_maps.append({
            "xlt": np.ascontiguousarray(xT[:, gathers[pair]]),
            "xrt": np.ascontiguousarray(xT[:, gathers[1 - pair]]),
            "wq": wqb, "wk": wkb, "wv": wvb,
            "masks": m[pair], "ones2": ones2,
        })
    return in_maps


def gather_output(results):
    out = np.empty((B, T, H), np.float32)
    for c in range(NCORES):
        b, pair = c // 2, c % 2
        r = np.asarray(results[c]["outp"], np.float32)
        for j in range(NLB):
            out[b, (2 * j + pair) * P:(2 * j + pair + 1) * P, :] = \
                r[j * P:(j + 1) * P, :]
    return out


_NC_CACHE = {}


def kernel(x, Wq, Wk, Wv):
    from concourse.bass_utils import run_bass_kernel_spmd

    x = np.asarray(x, dtype=np.float32)
    Wq = np.asarray(Wq, dtype=np.float32)
    Wk = np.asarray(Wk, dtype=np.float32)
    Wv = np.asarray(Wv, dtype=np.float32)

    if "nc" not in _NC_CACHE:
        _NC_CACHE["nc"] = build_module()
    nc = _NC_CACHE["nc"]

    in_maps = host_inputs(x, Wq, Wk, Wv)
    res = run_bass_kernel_spmd(nc, in_maps, core_ids=list(range(NCORES)))
    return gather_output(res.results)
